# revision 62
# baseline (speedup 1.0000x reference)
"""Trainium2 Bass kernel for nn_Batch_Edge (gnn_message_passing).

Computation (see reference):
    node_embed = last_node_batch @ W_embed + b_embed          # [B, H]
    stack      = concat([h, node_embed[seg]], axis=1)         # [N, 2H]
    out        = tanh(stack @ W1 + b1); out = tanh(out @ W2 + b2)
    edges      = out @ W3 + b3                                # [N, 2]
    result     = edges reshaped to [B, max_nodes*2]  (no padding: all graphs full)

Strategy: shard 512 graphs (131072 nodes) contiguously across 8 cores (64
graphs / 16384 nodes each).  Activations are feature-on-partition
([feature, node]); matmuls run in bf16 (1 col/cycle on the PE — fp32/f32r
run in the 4x-slower fp32 HIGH mode on TRN2 hardware).  The per-graph
embedding contribution C = node_embed @ W1[H:, :] + b1 is computed on the
host (tiny: B x 2H) and added into the L1 PSUM accumulation by K=3
indicator matmuls (per-chunk C rows x 0/1 graph indicator), so no
vector-engine broadcast adds are needed.  The chunk loop (20x768 nodes +
4x256-node drain chunks) is software-pipelined three stages deep —
iteration i issues L1 matmuls for chunk i, L2 for chunk i-1, and L3 for
chunk i-3 — so every tensor-engine instruction depends only on results
from previous iterations and the PE never stalls (which would re-throttle
the HAM clock gate).  The tanh layers ride ScalarE straight out of PSUM;
b2 rides the DVE's PSUM->SBUF evict; dummy warm-up matmuls during the
prologue DMA wait pre-warm the PE clock.  PSUM: z1 + z2 are 3-bank
tiles, L3 accumulates in a dedicated 2-tile pool, 8 banks total.
"""

import os
import numpy as np

B = 512
NPG = 256               # nodes per graph
N = B * NPG             # 131072
HID = 128
NCORES = 8
GPC = B // NCORES       # 64 graphs per core
NPC = N // NCORES       # 16384 nodes per core
PAD_VALUE = -10000.0

CH = 768                # nodes per full chunk (3 graphs)
NFULL = 20              # full chunks; the last 1024 nodes run as four
TAIL = 256              # 256-node chunks so the pipeline-drain chains
NTAIL = 4               # at the end of the kernel stay short
NCHUNK = NFULL + NTAIL

LAST_RESULT = None      # BassKernelResults of the most recent device run
_CACHE = {}


def _numpy_ref(last_node_batch, h, W_embed, b_embed, W1, b1, W2, b2, W3, b3,
               segment_ids, max_nodes):
    """Exact host fallback (used only if inputs don't match the expected
    uniform-graph structure)."""
    lnb = np.asarray(last_node_batch, np.float32)
    h = np.asarray(h, np.float32)
    seg = np.asarray(segment_ids).astype(np.int64)
    b = lnb.shape[0]
    n = h.shape[0]
    mn = int(np.asarray(max_nodes))
    node_embed = lnb @ np.asarray(W_embed, np.float32) + np.asarray(b_embed, np.float32)
    stack = np.concatenate([h, node_embed[seg]], axis=1)
    out = np.tanh(stack @ np.asarray(W1, np.float32) + np.asarray(b1, np.float32))
    out = np.tanh(out @ np.asarray(W2, np.float32) + np.asarray(b2, np.float32))
    edges = out @ np.asarray(W3, np.float32) + np.asarray(b3, np.float32)
    counts = np.zeros(b, np.int64)
    np.add.at(counts, seg, 1)
    offsets = np.cumsum(counts) - counts
    pos = np.arange(n) - offsets[seg]
    padded = np.full((b, mn, 2), PAD_VALUE, np.float32)
    padded[seg, pos] = edges
    return padded.reshape(b, mn * 2)


def _build():
    """Build + compile the per-core Bass program (identical on all cores)."""
    import concourse.bacc as bacc
    import concourse.mybir as mybir
    import concourse.tile as tile

    f32 = mybir.dt.float32
    bf16 = mybir.dt.bfloat16
    Tanh = mybir.ActivationFunctionType.Tanh

    nc = bacc.Bacc("TRN2", target_bir_lowering=False, debug=False, enable_asserts=False)

    # wpk columns (bf16): W1h[0:256] W2a[256:512] W2b[512:768] W3a[768:770]
    #                     W3b[770:772]; e3 is the graph-in-chunk indicator
    #                     (e3[j, c] = 1 iff c//256 == j), tiny and separate
    #                     so the prologue DMAs stay small.
    hT = nc.dram_tensor("hT", [128, NPC], bf16, kind="ExternalInput").ap()
    wpk = nc.dram_tensor("wpk", [128, 772], bf16, kind="ExternalInput").ap()
    # cols 0:768 = e3 indicator; cols 768+i*256+f = C[g0(i)+j, f] (chunk i)
    ct3 = nc.dram_tensor("ct3", [3, 768 + NCHUNK * 256], bf16,
                         kind="ExternalInput").ap()
    b2s = nc.dram_tensor("b2s", [128, 2], f32, kind="ExternalInput").ap()
    out_d = nc.dram_tensor("out", [2, NPC], f32, kind="ExternalOutput").ap()

    # Accumulation-group windows (node-offset, width) per (chunk-size, m).
    # Each group's PSUM output must stay inside one 2KB bank (512 fp32 cols,
    # bank boundaries at z-tile cols 512/1024), so the split differs between
    # the m0 span (cols 0:768) and the m1 span (cols 768:1536).
    def windows(ch, m):
        if ch == CH:
            return [(0, 512), (512, 256)] if m == 0 else [(0, 256), (256, 512)]
        return [(0, 256)]

    with tile.TileContext(nc) as tc:
        with (
            tc.tile_pool(name="w", bufs=1) as wp,
            tc.tile_pool(name="io", bufs=3) as io,
            tc.tile_pool(name="act", bufs=2) as ac,
            tc.tile_pool(name="act3", bufs=3) as ac3,
            tc.tile_pool(name="ps12", bufs=1, space="PSUM") as ps12,
            tc.tile_pool(name="ps3", bufs=2, space="PSUM") as ps3,
        ):
            s_w = wp.tile([128, 772], bf16, tag="wpk")
            s_ctA = wp.tile([3, 768 + NCHUNK * 256], bf16, tag="ct3")
            s_ct = s_ctA[:, 768:]

            s_w1 = s_w[:, 0:256]
            s_E3 = s_ctA[:, 0:768]
            s_w2a = s_w[:, 256:512]
            s_w2b = s_w[:, 512:768]
            s_w3a = s_w[:, 768:770]
            s_w3b = s_w[:, 770:772]

            chunks = ([(i * CH, CH) for i in range(NFULL)]
                      + [(NFULL * CH + j * TAIL, TAIL) for j in range(NTAIL)])
            # column offset of the m-half within the z tiles (and y tiles)
            mbase = {CH: (0, 768), TAIL: (0, 512)}

            h_tiles = {}
            z1_t = {}
            y1_t = {}
            y2_t = {}
            z2_t = {}
            p3_t = {}

            def dma_h(i, split=False):
                base, ch = chunks[i]
                t = io.tile([128, CH], bf16, tag="h")
                if split:
                    # two DMA queues halve the first chunk's arrival latency
                    nc.sync.dma_start(out=t[:, 0:ch // 2],
                                      in_=hT[:, base:base + ch // 2])
                    nc.sync.dma_start(out=t[:, ch // 2:ch],
                                      in_=hT[:, base + ch // 2:base + ch])
                else:
                    nc.sync.dma_start(out=t[:, 0:ch], in_=hT[:, base:base + ch])
                h_tiles[i] = t

            def l1mm(i):
                base, ch = chunks[i]
                t_h = h_tiles.pop(i)
                z1 = ps12.tile([128, 1536], f32, tag="z1")
                # Every matmul output must stay inside one PSUM bank (ISA
                # limit), and bank1 is shared between the m0 and m1 spans,
                # so m0's groups fully close before m1's open.  Within each
                # m-half the h matmuls batch before the C-add indicator
                # matmuls to keep LDWEIGHTS pull-ahead effective.
                for m in (0, 1):
                    mb = mbase[ch][m]
                    for (o, w) in windows(ch, m):
                        nc.tensor.matmul(
                            z1[:, mb + o:mb + o + w],
                            s_w1[:, m * 128:m * 128 + 128],
                            t_h[:, o:o + w],
                            start=True, stop=False,
                        )
                    for (o, w) in windows(ch, m):
                        nc.tensor.matmul(
                            z1[:, mb + o:mb + o + w],
                            s_ct[:, i * 256 + m * 128:i * 256 + m * 128 + 128],
                            s_E3[:, o:o + w],
                            start=False, stop=True,
                        )
                z1_t[i] = z1

            def act1(i):
                base, ch = chunks[i]
                z1 = z1_t.pop(i)
                # tanh over the full span (tail: middle 256 cols are unused
                # garbage, harmless)
                span = mbase[ch][1] + ch
                y1 = ac.tile([128, 1536], bf16, tag="y1")
                nc.scalar.activation(y1[:, 0:span], z1[:, 0:span], Tanh)
                y1_t[i] = y1

            def l2mm(i):
                base, ch = chunks[i]
                y1 = y1_t.pop(i)
                mb1 = mbase[ch][1]
                z2 = ps12.tile([128, 1536], f32, tag="z2")
                for m in (0, 1):
                    mb = mbase[ch][m]
                    for (o, w) in windows(ch, m):
                        nc.tensor.matmul(
                            z2[:, mb + o:mb + o + w],
                            s_w2a[:, m * 128:m * 128 + 128],
                            y1[:, o:o + w],
                            start=True, stop=False,
                        )
                    for (o, w) in windows(ch, m):
                        nc.tensor.matmul(
                            z2[:, mb + o:mb + o + w],
                            s_w2b[:, m * 128:m * 128 + 128],
                            y1[:, mb1 + o:mb1 + o + w],
                            start=False, stop=True,
                        )
                z2_t[i] = z2

            def l2post(i):
                base, ch = chunks[i]
                z2 = z2_t.pop(i)
                mb1 = mbase[ch][1]
                y2s = ac.tile([128, 1536], bf16, tag="y2s")
                nc.vector.tensor_scalar_add(y2s[:, 0:ch], z2[:, 0:ch], s_b2[:, 0:1])
                nc.vector.tensor_scalar_add(y2s[:, mb1:mb1 + ch], z2[:, mb1:mb1 + ch],
                                            s_b2[:, 1:2])
                span = mb1 + ch
                y2 = ac3.tile([128, 1536], bf16, tag="y2")
                nc.scalar.activation(y2[:, 0:span], y2s[:, 0:span], Tanh)
                y2_t[i] = y2

            def l3mm(i):
                base, ch = chunks[i]
                y2 = y2_t.pop(i)
                mb1 = mbase[ch][1]
                tiles = []
                for (o, w) in windows(ch, 0):
                    p = ps3.tile([2, 512], f32, tag="p3")
                    nc.tensor.matmul(p[:, 0:w], s_w3a, y2[:, o:o + w],
                                     start=True, stop=False)
                    nc.tensor.matmul(p[:, 0:w], s_w3b, y2[:, mb1 + o:mb1 + o + w],
                                     start=False, stop=True)
                    tiles.append((p, o, w))
                p3_t[i] = tiles

            def evict(i):
                base, ch = chunks[i]
                tiles = p3_t.pop(i)
                ed = io.tile([2, CH], f32, tag="ed")
                for (p, o, w) in tiles:
                    nc.vector.tensor_copy(ed[:, o:o + w], p[:, 0:w])
                nc.sync.dma_start(out=out_d[:, base:base + ch], in_=ed[:, 0:ch])

            # 4-deep software pipeline: iteration i runs L2 matmuls for chunk
            # i-1 and L3 matmuls for chunk i-3 BEFORE the chunk-i L1 matmuls,
            # so every tensor-engine instruction depends only on activations
            # issued in earlier iterations and the PE never idles.
            # PE warm-up: the HAM clock gate holds the PE at 1.2 GHz until it
            # has been busy for a ~3.4us activity window.  The tensor engine
            # is idle waiting for the prologue DMAs anyway, so a burst of
            # dummy matmuls on a zeroed scratch tile brings it to 2.4 GHz
            # before the first real matmul, off the critical path.
            g_w = wp.tile([128, 512], bf16, tag="gw")
            nc.vector.memset(g_w[:], 0.0)
            g_p = ps3.tile([2, 512], f32, tag="p3")
            for _ in range(10):
                nc.tensor.matmul(g_p[:], g_w[:, 0:2], g_w[:, 0:512],
                                 start=True, stop=True)

            # Prologue DMAs ordered by when the first L1 matmuls need them;
            # each dma_start costs ~0.6us of sync-engine issue time and a
            # single queue moves ~50GB/s, so the critical h0/W1/e3/ct3 chain
            # is split small and first, everything else after.
            dma_h(0, split=True)
            nc.sync.dma_start(out=s_ctA[:], in_=ct3)
            nc.sync.dma_start(out=s_w[:, 0:256], in_=wpk[:, 0:256])
            nc.sync.dma_start(out=s_w[:, 256:772], in_=wpk[:, 256:772])
            s_b2 = wp.tile([128, 2], f32, tag="b2s")
            nc.sync.dma_start(out=s_b2[:], in_=b2s)
            dma_h(1)
            for i in range(NCHUNK + 3):
                if i + 2 < NCHUNK:
                    dma_h(i + 2)
                if i < NCHUNK:
                    l1mm(i)
                    act1(i)
                if 1 <= i <= NCHUNK:
                    l2mm(i - 1)
                if 3 <= i <= NCHUNK + 2:
                    l3mm(i - 3)
                if 1 <= i <= NCHUNK:
                    l2post(i - 1)
                if 3 <= i <= NCHUNK + 2:
                    evict(i - 3)

    nc.compile()
    return nc


def _to_bf16(x):
    import ml_dtypes
    return np.ascontiguousarray(x.astype(ml_dtypes.bfloat16))


def kernel(last_node_batch, h, W_embed, b_embed, W1, b1, W2, b2, W3, b3,
           segment_ids, max_nodes):
    global LAST_RESULT
    lnb = np.asarray(last_node_batch, np.float32)
    h = np.asarray(h, np.float32)
    seg = np.asarray(segment_ids)
    mn = int(np.asarray(max_nodes))

    expected_seg = np.repeat(np.arange(B, dtype=seg.dtype), NPG)
    if not (lnb.shape == (B, HID) and h.shape == (N, HID) and mn == NPG
            and seg.shape == (N,) and np.array_equal(seg, expected_seg)):
        return _numpy_ref(last_node_batch, h, W_embed, b_embed, W1, b1, W2, b2,
                          W3, b3, segment_ids, max_nodes)

    import sys
    try:
        import antenv.axon_hooks  # noqa: F401
    except ImportError:
        # bass_utils imports this unconditionally when tracing is requested
        # (e.g. BASS_TRACE set in the environment); provide a no-op fallback
        # so tracing degrades instead of crashing.
        import types
        _m = types.ModuleType("antenv.axon_hooks")
        _m.get_axon_ntff_profile_hook = lambda: None
        _m.set_axon_ntff_profile_hook = lambda h: None
        sys.modules["antenv.axon_hooks"] = _m

    from concourse.bass_utils import run_bass_kernel_spmd

    if "nc" not in _CACHE:
        _CACHE["nc"] = _build()
    nc = _CACHE["nc"]

    W1 = np.asarray(W1, np.float32)
    W2 = np.asarray(W2, np.float32)
    W3 = np.asarray(W3, np.float32)
    b1v = np.asarray(b1, np.float32)
    b2v = np.asarray(b2, np.float32)
    b3v = np.asarray(b3, np.float32)

    # per-graph L1 contribution: C = (lnb @ W_embed + b_embed) @ W1[H:] + b1
    ne = lnb @ np.asarray(W_embed, np.float32) + np.asarray(b_embed, np.float32)
    C = ne @ W1[HID:, :] + b1v                                  # [B, 2H]

    E3 = np.zeros((3, 768), np.float32)
    for j in range(3):
        E3[j, j * 256:(j + 1) * 256] = 1.0
    wpk = _to_bf16(np.concatenate(
        [W1[:HID, :], W2[:HID, :], W2[HID:, :], W3[:HID, :], W3[HID:, :]],
        axis=1))
    b2s = np.ascontiguousarray(np.stack([b2v[:HID], b2v[HID:]], axis=1))

    # graph index of each chunk's first graph (20 x 768-node + 4 x 256-node)
    gbase = [3 * i for i in range(NFULL)] + [3 * NFULL + j for j in range(NTAIL)]

    in_maps = []
    for c in range(NCORES):
        # ct3 = [e3 indicator | per-chunk C rows], zero-padded
        ct3 = np.zeros((3, 768 + NCHUNK * 256), np.float32)
        ct3[:, 0:768] = E3
        Cc = C[c * GPC:(c + 1) * GPC]
        for i, g0 in enumerate(gbase):
            ng = 3 if i < NFULL else 1
            for j in range(ng):
                ct3[j, 768 + i * 256:768 + (i + 1) * 256] = Cc[g0 + j]
        m = {
            "wpk": wpk,
            "ct3": _to_bf16(ct3),
            "b2s": b2s,
            "hT": _to_bf16(np.ascontiguousarray(h[c * NPC:(c + 1) * NPC].T)),
        }
        in_maps.append(m)

    trace = bool(int(os.environ.get("KERNEL_TRACE", "0")))
    res = run_bass_kernel_spmd(nc, in_maps, core_ids=list(range(NCORES)),
                               trace=trace)
    LAST_RESULT = res

    out = np.empty((B, NPG * 2), np.float32)
    for c in range(NCORES):
        od = res.results[c]["out"]          # [2, NPC]; [cc, n] = edges[n, cc]
        blk = od.reshape(2, GPC, NPG).transpose(1, 2, 0).reshape(GPC, NPG * 2)
        out[c * GPC:(c + 1) * GPC] = blk
    out += np.tile(b3v, NPG)[None, :]
    return out
